# revision 12
# baseline (speedup 1.0000x reference)
"""AWPLoss kernel for Trainium2 (8 NeuronCores, pure data-parallel over batch).

Reference semantics (nn_AWPLoss): sample an alignment a ~ Categorical(log_probs)
per (b, t), clone it (f_prop = identity), and compute
    loss = mean(relu(lambda + log_probs[b,t,a] - log_probs[b,t,a_clone])).
Because the alignment is cloned, original_prob and enhanced_prob are the same
tensor, and the loss reduces to mean(relu(fl(lambda + p) - p)) where p is the
log-prob of the chosen class — the value depends on the sample only through
float32 rounding of (lambda + p) - p, i.e. at the ~1e-5 relative level.

This kernel therefore streams all of log_probs through SBUF (the memory
roofline for this problem), takes the greedy sample p = max_c log_probs[b,t,c]
per row (the mode of the categorical — any choice of sample agrees with the
reference to ~2e-5 relative), computes relu((lambda + p) - p) in float32, and
accumulates. Batch B=64 is sharded 8 ways; per-core partial sums are combined
on the host.

Per-core layout: shard [8, 4096, 128] viewed flat as [32768 rows, 128 classes].
Partition p of SBUF owns rows [p*256, (p+1)*256); each tile moves RT rows per
partition (contiguous RT*512 bytes per partition per DMA).
"""

import numpy as np

B, T, C = 64, 4096, 128
N_CORES = 8
B_PER_CORE = B // N_CORES            # 8
ROWS_PER_CORE = B_PER_CORE * T       # 32768
ROWS_PER_PART = ROWS_PER_CORE // 128  # 256 rows owned by each SBUF partition
RT = 32                              # rows per partition per tile (2 MiB tiles)
N_TILES = ROWS_PER_PART // RT
LAMBDA = 0.01
PIPE_DEPTH = 2  # stream DMAs allowed in flight

_NC_CACHE = {}


def _build_bass():
    import concourse.bass as bass
    import concourse.mybir as mybir
    from concourse.tile import TileContext

    nc = bass.Bass()
    x = nc.dram_tensor(
        "x", [ROWS_PER_CORE, C], mybir.dt.float32, kind="ExternalInput"
    )
    partial = nc.dram_tensor(
        "partial", [128, RT], mybir.dt.float32, kind="ExternalOutput"
    )

    # [128, ROWS_PER_PART*C]: partition p's line = rows p*256..(p+1)*256 flat.
    xv = x[:, :].rearrange("(p b) c -> p (b c)", p=128)

    from bass_rust import add_dep_helper

    with TileContext(nc) as tc:
        with tc.tile_pool(name="pool", bufs=1) as pool:
            acc = pool.tile([128, RT], mybir.dt.float32, tag="acc")
            nc.vector.memset(acc[:, :], 0.0)
            # Each tile gets its own SBUF slot (no reuse): a reused slot makes
            # Tile attach two sync waits (WAR + WAW) to the stream DMA, and
            # walrus codegen allows only one wait per DMACopy. Pipelining is
            # instead enforced with one explicit dep per DMA (on the reduce
            # two tiles back) so ~2 loads are in flight and DVE overlaps.
            reduces = []
            dmas = []
            last_acc = None
            for t in range(N_TILES):
                tile = pool.tile([128, RT * C], mybir.dt.float32, tag=f"s{t}")
                dma = nc.sync.dma_start(
                    out=tile[:, :], in_=xv[:, t * RT * C : (t + 1) * RT * C]
                )
                dmas.append(dma)
                if t >= PIPE_DEPTH:
                    add_dep_helper(
                        dma.ins,
                        reduces[t - PIPE_DEPTH].ins,
                        reason="throttle stream DMA issue",
                    )
                pmax = pool.tile([128, RT], mybir.dt.float32, tag=f"m{t}")
                red = nc.vector.reduce_max(
                    out=pmax[:, :],
                    in_=tile[:, :].rearrange("p (r c) -> p r c", c=C),
                    axis=mybir.AxisListType.X,
                )
                reduces.append(red)
                d = pool.tile([128, RT], mybir.dt.float32, tag=f"d{t}")
                # d = (pmax + LAMBDA) - pmax, in float32, matching the
                # reference's (LAMBDA + p) - p evaluation order.
                nc.vector.scalar_tensor_tensor(
                    out=d[:, :],
                    in0=pmax[:, :],
                    scalar=LAMBDA,
                    in1=pmax[:, :],
                    op0=mybir.AluOpType.add,
                    op1=mybir.AluOpType.subtract,
                )
                # acc += relu(d)
                last_acc = nc.vector.scalar_tensor_tensor(
                    out=acc[:, :],
                    in0=d[:, :],
                    scalar=0.0,
                    in1=acc[:, :],
                    op0=mybir.AluOpType.max,
                    op1=mybir.AluOpType.add,
                )
            # SWDGE for the store: a 9th HWDGE DMA would reuse a DMAHW lane
            # and pick up a lane-reuse wait on top of its DVE wait (walrus
            # allows only one wait per DMACopy).
            store = nc.gpsimd.dma_start(out=partial[:, :], in_=acc[:, :])
            # Absorber nops on SP: Tile's kernel-tail drain (on SP) waits on
            # every proc's final tick, and walrus caps sync waits per
            # instruction. Observing each completion on SP beforehand — one
            # wait per nop — lets add_sem_waits elide them all on the drain.
            for dep in [*dmas, store, last_acc]:
                nop = nc.sync.nop(nofuse=True, hint="drain_absorb")
                add_dep_helper(nop.ins, dep.ins, reason="absorb drain wait")
    return nc


def _get_nc():
    if "nc" not in _NC_CACHE:
        _NC_CACHE["nc"] = _build_bass()
    return _NC_CACHE["nc"]


def _run(lp, trace=False):
    from concourse.bass_utils import run_bass_kernel_spmd

    in_maps = [
        {"x": np.ascontiguousarray(lp[c * B_PER_CORE : (c + 1) * B_PER_CORE]).reshape(
            ROWS_PER_CORE, C
        )}
        for c in range(N_CORES)
    ]
    return run_bass_kernel_spmd(
        _get_nc(), in_maps, core_ids=list(range(N_CORES)), trace=trace
    )


def kernel(log_probs, targets=None, input_lengths=None, target_lengths=None):
    lp = np.asarray(log_probs, dtype=np.float32)
    assert lp.shape == (B, T, C), lp.shape
    res = _run(lp)
    total = sum(r["partial"].sum(dtype=np.float64) for r in res.results)
    return np.asarray(total / (B * T), dtype=np.float32)
